# revision 23
# baseline (speedup 1.0000x reference)
"""AlphaKnotLoss on 8 TRN2 NeuronCores (Bass/Tile, SPMD data-parallel).

Reference computation (B=4096 graphs x 512 nodes x A=10 actions):
  loss_val    = mean((values - target_vals)^2)                  over B
  per graph g: Z[g]   = sum_{n in g, a} exp(logits[n,a])
               Lin[g] = sum_{n in g, a} target_probs[n,a]*logits[n,a]
               V[g]   = sum_{n in g, a} target_probs[n,a]
               lp[g]  = (log(Z[g]+eps) - Lin[g]) / (V[g]+eps)
  out = loss_val + mean(lp)

Sharding: data-parallel over graphs. Each of the 8 cores owns 512 whole
graphs = 262144 nodes. Per-core layout: the (262144, 10) node-major shard
is viewed as (128, 20480) so partition p holds 4 whole graphs
(4 x 512 nodes x 10 actions = 20480 contiguous floats); a graph is a
5120-wide contiguous block of the free axis, split across NT tiles.

Per tile (128 x F): ACT does exp with fused accumulate (Z) and a
copy-accumulate (V); DVE does one fused multiply+reduce via
scalar_tensor_tensor (Lin). The loop is DMA-bound (~56us of HBM traffic
per core); ACT ~37us and DVE ~22us hide underneath it.

Epilogue: per-graph losses on (128, GPP) stats, one PE matmul against a
ones vector for the cross-partition sum, then either a tiny AllReduce
(mode "allreduce": every core returns the final scalar) or per-core
partial sums combined on the host (mode "partials", default: avoids the
collective's mesh-entry barrier, which couples every core to the
slowest/last-started core).
"""

import numpy as np

B = 4096
NPG = 512
N = B * NPG
A = 10
EPS = 1e-9
M = 8  # cores

NC_NODES = N // M          # 262144 nodes per core
NC_GRAPHS = B // M         # 512 graphs per core
P = 128                    # SBUF partitions
FREE = NC_NODES * A // P   # 20480 f32 per partition
GPP = NC_GRAPHS // P       # 4 graphs per partition
GFREE = NPG * A            # 5120 f32 per graph

# Per-graph tile widths (must sum to GFREE). A thin final slice keeps the
# critical path after the last DMA byte short.
GRAPH_TILES = [2560, 2560]
TPG = len(GRAPH_TILES)
NT = GPP * TPG
# When True, the very last graph column uses widths [2560, 1280, 1280] so the
# final DVE/ACT ops after the last DMA byte are half-width.
SPLIT_LAST = True

IO_BUFS = 3
WORK_BUFS = 3

# False: partials leave via a PE ones-matmul -> PSUM[1,8] -> SBUF -> DRAM.
# True: DMA the per-partition S[128,8] straight out; host reduces.
# A/B (20 reps each): direct-out floor 72.3us vs 68.5us — the 128-row
# 32B-chunk output DMA costs more than the matmul chain. Keep False.
DIRECT_OUT = False

# "lfirst": stream the whole logits shard first (kept resident in SBUF,
# exp/log(Z) finish mid-stream), then stream probs; the post-last-byte
# chain shrinks to one half-width stt + copy + the scalar epilogue.
LFIRST = False


def set_params(graph_tiles=None, io_bufs=None, work_bufs=None, mode=None,
               split_last=None, lfirst=None, direct_out=None):
    """Tweak build knobs (test harness only); clears the build cache."""
    global GRAPH_TILES, TPG, NT, IO_BUFS, WORK_BUFS, MODE, SPLIT_LAST, LFIRST
    global DIRECT_OUT
    if split_last is not None:
        SPLIT_LAST = split_last
    if lfirst is not None:
        LFIRST = lfirst
    if direct_out is not None:
        DIRECT_OUT = direct_out
    if graph_tiles is not None:
        assert sum(graph_tiles) == GFREE
        GRAPH_TILES = list(graph_tiles)
        TPG = len(GRAPH_TILES)
        NT = GPP * TPG
    if io_bufs is not None:
        IO_BUFS = io_bufs
    if work_bufs is not None:
        WORK_BUFS = work_bufs
    if mode is not None:
        MODE = mode
    _CACHE.clear()

MODE = "partials"

_CACHE = {}


def _build(mode):
    import concourse.bacc as bacc
    import concourse.mybir as mybir
    import concourse.tile as tile

    f32 = mybir.dt.float32
    Alu = mybir.AluOpType
    Act = mybir.ActivationFunctionType
    AX = mybir.AxisListType.X

    nc = bacc.Bacc("TRN2", target_bir_lowering=False, debug=False,
                   num_devices=M)

    logits = nc.dram_tensor("logits", [P, FREE], f32, kind="ExternalInput")
    probs = nc.dram_tensor("probs", [P, FREE], f32, kind="ExternalInput")
    vals = nc.dram_tensor("vals", [P, GPP], f32, kind="ExternalInput")
    tvals = nc.dram_tensor("tvals", [P, GPP], f32, kind="ExternalInput")
    if mode == "allreduce" or not DIRECT_OUT:
        out = nc.dram_tensor("out", [1, 8], f32, kind="ExternalOutput")
    else:
        # per-partition partials go out directly; host does the final
        # 128x2-per-core reduction (no PE/PSUM in the graph at all)
        out = nc.dram_tensor("out", [P, 8], f32, kind="ExternalOutput")

    import contextlib

    with tile.TileContext(nc) as tc:
        with contextlib.ExitStack() as ctx:
            iop = ctx.enter_context(tc.tile_pool(name="io", bufs=IO_BUFS))
            wp = ctx.enter_context(tc.tile_pool(name="work", bufs=WORK_BUFS))
            sp = ctx.enter_context(tc.tile_pool(name="stats", bufs=1))
            if mode == "allreduce" or not DIRECT_OUT:
                pp = ctx.enter_context(
                    tc.tile_pool(name="psum", bufs=1, space="PSUM"))
            if mode == "allreduce":
                dp = ctx.enter_context(
                    tc.tile_pool(name="dram", bufs=1, space="DRAM"))
            # per-tile widths for each graph column; optionally split the
            # tail of the last graph for a shorter post-DMA chain
            widths = []
            for g in range(GPP):
                if SPLIT_LAST and g == GPP - 1:
                    widths += [GRAPH_TILES[0]]
                    rest = GFREE - GRAPH_TILES[0]
                    widths += [rest // 2, rest - rest // 2]
                else:
                    widths += list(GRAPH_TILES)
            ncols = len(widths)
            Z = sp.tile([P, ncols], f32)
            Lin = sp.tile([P, ncols], f32)
            V = sp.tile([P, ncols], f32)

            fmax = max(GRAPH_TILES)
            off = 0
            for j in range(ncols):
                w = widths[j]
                lt = iop.tile([P, fmax], f32, tag="lt")
                pt = iop.tile([P, fmax], f32, tag="pt")
                et = wp.tile([P, fmax], f32, tag="et")
                cp = wp.tile([P, fmax], f32, tag="cp")
                prod = wp.tile([P, fmax], f32, tag="prod")
                nc.sync.dma_start(lt[:, :w], logits[:, off:off + w])
                nc.sync.dma_start(pt[:, :w], probs[:, off:off + w])
                nc.scalar.activation(et[:, :w], lt[:, :w], Act.Exp,
                                     accum_out=Z[:, j:j + 1])
                nc.scalar.activation(cp[:, :w], pt[:, :w], Act.Copy,
                                     accum_out=V[:, j:j + 1])
                nc.vector.scalar_tensor_tensor(
                    out=prod[:, :w], in0=lt[:, :w], scalar=1.0,
                    in1=pt[:, :w], op0=Alu.mult, op1=Alu.mult,
                    accum_out=Lin[:, j:j + 1])
                off += w

            # reduce tile-partials down to GPP per-graph sums
            nuni = (GPP - 1) * TPG if SPLIT_LAST else GPP * TPG
            Zg = sp.tile([P, GPP], f32)
            Lg = sp.tile([P, GPP], f32)
            Vg = sp.tile([P, GPP], f32)
            for src, dst in ((Z, Zg), (Lin, Lg), (V, Vg)):
                if SPLIT_LAST:
                    nc.vector.reduce_sum(
                        dst[:, 0:GPP - 1],
                        src[:, 0:nuni].rearrange("p (g t) -> p g t", t=TPG),
                        axis=AX)
                    nc.vector.reduce_sum(
                        dst[:, GPP - 1:GPP], src[:, nuni:ncols], axis=AX)
                else:
                    nc.vector.reduce_sum(
                        dst[:, :],
                        src[:, :].rearrange("p (g t) -> p g t", t=TPG),
                        axis=AX)

            # per-graph policy loss: (ln(Z+eps) - Lin) / (V+eps)
            zp = sp.tile([P, GPP], f32)
            nc.vector.tensor_scalar_add(zp[:, :], Zg[:, :], EPS)
            logz = sp.tile([P, GPP], f32)
            nc.scalar.activation(logz[:, :], zp[:, :], Act.Ln)
            num = sp.tile([P, GPP], f32)
            nc.vector.tensor_sub(num[:, :], logz[:, :], Lg[:, :])
            den = sp.tile([P, GPP], f32)
            nc.vector.tensor_scalar_add(den[:, :], Vg[:, :], EPS)
            rec = sp.tile([P, GPP], f32)
            nc.vector.reciprocal(rec[:, :], den[:, :])

            # S[:,0] = per-partition policy sum, S[:,1] = value-sq sum
            S = sp.tile([P, 8], f32)
            nc.gpsimd.memset(S[:, :], 0.0)
            lp = sp.tile([P, GPP], f32)
            nc.vector.scalar_tensor_tensor(
                out=lp[:, :], in0=num[:, :], scalar=1.0, in1=rec[:, :],
                op0=Alu.mult, op1=Alu.mult, accum_out=S[:, 0:1])

            vt = sp.tile([P, GPP], f32)
            tt = sp.tile([P, GPP], f32)
            nc.sync.dma_start(vt[:, :], vals[:, :])
            nc.sync.dma_start(tt[:, :], tvals[:, :])
            d = sp.tile([P, GPP], f32)
            nc.vector.tensor_sub(d[:, :], vt[:, :], tt[:, :])
            d2 = sp.tile([P, GPP], f32)
            nc.vector.scalar_tensor_tensor(
                out=d2[:, :], in0=d[:, :], scalar=1.0, in1=d[:, :],
                op0=Alu.mult, op1=Alu.mult, accum_out=S[:, 1:2])

            if mode == "allreduce" or not DIRECT_OUT:
                # cross-partition sum via matmul with a ones vector
                ones = sp.tile([P, 1], f32)
                nc.gpsimd.memset(ones[:, :], 1.0)
                ps = pp.tile([1, 8], f32)
                nc.tensor.matmul(ps[:, :], ones[:, :], S[:, :],
                                 start=True, stop=True)
                red = sp.tile([1, 8], f32)
                nc.vector.tensor_copy(red[:, :], ps[:, :])
            if mode == "allreduce":
                cin = dp.tile([1, 8], f32)
                cout = dp.tile([1, 8], f32)
                nc.sync.dma_start(cin[:, :], red[:, :])
                nc.gpsimd.collective_compute(
                    "AllReduce", Alu.add,
                    replica_groups=[list(range(M))],
                    ins=[cin[:, :].opt()],
                    outs=[cout[:, :].opt()])
                red2 = sp.tile([1, 8], f32)
                nc.sync.dma_start(red2[:, :], cout[:, :])
                # out = (sum_policy + sum_val) / B
                dummy = sp.tile([1, 2], f32)
                fin = sp.tile([1, 8], f32)
                nc.gpsimd.memset(fin[:, :], 0.0)
                nc.scalar.activation(dummy[:, :], red2[:, 0:2], Act.Copy,
                                     scale=1.0 / B, accum_out=fin[:, 0:1])
                nc.sync.dma_start(out[:, :], fin[:, :])
            elif DIRECT_OUT:
                nc.sync.dma_start(out[:, :], S[:, :])
            else:
                nc.sync.dma_start(out[:, :], red[:, :])

    nc.compile()
    return nc


def _build_lfirst(mode):
    import concourse.bacc as bacc
    import concourse.mybir as mybir
    import concourse.tile as tile

    f32 = mybir.dt.float32
    Alu = mybir.AluOpType
    Act = mybir.ActivationFunctionType
    AX = mybir.AxisListType.X

    nc = bacc.Bacc("TRN2", target_bir_lowering=False, debug=False,
                   num_devices=M)

    logits = nc.dram_tensor("logits", [P, FREE], f32, kind="ExternalInput")
    probs = nc.dram_tensor("probs", [P, FREE], f32, kind="ExternalInput")
    vals = nc.dram_tensor("vals", [P, GPP], f32, kind="ExternalInput")
    tvals = nc.dram_tensor("tvals", [P, GPP], f32, kind="ExternalInput")
    out = nc.dram_tensor("out", [1, 8], f32, kind="ExternalOutput")

    LW = 2560                    # logits tile width
    LNT = FREE // LW             # 8 resident logits tiles
    # probs widths: uniform except the last graph's tail is split in half
    pw = [LW] * (LNT - 1) + [LW // 2, LW // 2]
    PNT = len(pw)                # 9
    LEAD = 3                     # logits tiles ahead of probs in the stream

    with tile.TileContext(nc) as tc:
        with (
            tc.tile_pool(name="lres", bufs=LNT) as lrp,
            tc.tile_pool(name="pio", bufs=IO_BUFS) as pip_,
            tc.tile_pool(name="work", bufs=WORK_BUFS) as wp,
            tc.tile_pool(name="stats", bufs=1) as sp,
            tc.tile_pool(name="psum", bufs=1, space="PSUM") as pp,
        ):
            Z = sp.tile([P, LNT], f32)
            V = sp.tile([P, PNT], f32)
            Lin = sp.tile([P, PNT], f32)

            ltiles = [lrp.tile([P, LW], f32, tag="lt", name=f"lt{j}")
                      for j in range(LNT)]

            def emit_logits(j):
                nc.sync.dma_start(ltiles[j][:, :],
                                  logits[:, j * LW:(j + 1) * LW])
                et = wp.tile([P, LW], f32, tag="et", name=f"et{j}")
                nc.scalar.activation(et[:, :], ltiles[j][:, :], Act.Exp,
                                     accum_out=Z[:, j:j + 1])

            poff = [0]

            def emit_probs(j):
                w = pw[j]
                off = poff[0]
                pt = pip_.tile([P, LW], f32, tag="pt", name=f"pt{j}")
                nc.sync.dma_start(pt[:, :w], probs[:, off:off + w])
                lsrc = ltiles[off // LW][:, off % LW:off % LW + w]
                cp = wp.tile([P, LW], f32, tag="cp", name=f"cp{j}")
                nc.scalar.activation(cp[:, :w], pt[:, :w], Act.Copy,
                                     accum_out=V[:, j:j + 1])
                prod = wp.tile([P, LW], f32, tag="prod", name=f"prod{j}")
                nc.vector.scalar_tensor_tensor(
                    out=prod[:, :w], in0=lsrc, scalar=1.0, in1=pt[:, :w],
                    op0=Alu.mult, op1=Alu.mult,
                    accum_out=Lin[:, j:j + 1])
                poff[0] += w

            # interleaved stream: logits LEAD tiles ahead so exp/log(Z)
            # finish before the probs stream ends
            li = pi = 0
            for j in range(LEAD):
                emit_logits(li)
                li += 1
            while li < LNT:
                emit_probs(pi)
                pi += 1
                emit_logits(li)
                li += 1
            # log(Z+eps) per graph — scheduled right after the last exp,
            # well before the stream ends
            Zg = sp.tile([P, GPP], f32)
            nc.vector.reduce_sum(
                Zg[:, :], Z[:, :].rearrange("p (g t) -> p g t", t=2),
                axis=AX)
            zp = sp.tile([P, GPP], f32)
            nc.vector.tensor_scalar_add(zp[:, :], Zg[:, :], EPS)
            logz = sp.tile([P, GPP], f32)
            nc.scalar.activation(logz[:, :], zp[:, :], Act.Ln)
            while pi < PNT:
                emit_probs(pi)
                pi += 1

            # per-graph sums: graphs 0..2 from column pairs, graph 3 from
            # the last three columns
            Vg = sp.tile([P, GPP], f32)
            Lg = sp.tile([P, GPP], f32)
            for src, dst in ((V, Vg), (Lin, Lg)):
                nc.vector.reduce_sum(
                    dst[:, 0:GPP - 1],
                    src[:, 0:2 * (GPP - 1)].rearrange(
                        "p (g t) -> p g t", t=2),
                    axis=AX)
                nc.vector.reduce_sum(dst[:, GPP - 1:GPP],
                                     src[:, 2 * (GPP - 1):PNT], axis=AX)

            den = sp.tile([P, GPP], f32)
            nc.vector.tensor_scalar_add(den[:, :], Vg[:, :], EPS)
            rec = sp.tile([P, GPP], f32)
            nc.vector.reciprocal(rec[:, :], den[:, :])
            num = sp.tile([P, GPP], f32)
            nc.vector.tensor_sub(num[:, :], logz[:, :], Lg[:, :])

            S = sp.tile([P, 8], f32)
            nc.gpsimd.memset(S[:, :], 0.0)
            lp = sp.tile([P, GPP], f32)
            nc.vector.scalar_tensor_tensor(
                out=lp[:, :], in0=num[:, :], scalar=1.0, in1=rec[:, :],
                op0=Alu.mult, op1=Alu.mult, accum_out=S[:, 0:1])

            vt = sp.tile([P, GPP], f32)
            tt = sp.tile([P, GPP], f32)
            nc.sync.dma_start(vt[:, :], vals[:, :])
            nc.sync.dma_start(tt[:, :], tvals[:, :])
            d = sp.tile([P, GPP], f32)
            nc.vector.tensor_sub(d[:, :], vt[:, :], tt[:, :])
            d2 = sp.tile([P, GPP], f32)
            nc.vector.scalar_tensor_tensor(
                out=d2[:, :], in0=d[:, :], scalar=1.0, in1=d[:, :],
                op0=Alu.mult, op1=Alu.mult, accum_out=S[:, 1:2])

            ones = sp.tile([P, 1], f32)
            nc.gpsimd.memset(ones[:, :], 1.0)
            ps = pp.tile([1, 8], f32)
            nc.tensor.matmul(ps[:, :], ones[:, :], S[:, :],
                             start=True, stop=True)
            red = sp.tile([1, 8], f32)
            nc.vector.tensor_copy(red[:, :], ps[:, :])
            nc.sync.dma_start(out[:, :], red[:, :])

    nc.compile()
    return nc


def _get(mode):
    if mode not in _CACHE:
        _CACHE[mode] = (_build_lfirst(mode) if LFIRST and mode == "partials"
                        else _build(mode))
    return _CACHE[mode]


def _make_in_maps(logits, values, target_probs, target_vals):
    in_maps = []
    lg = logits.reshape(M, P, FREE)
    pg = target_probs.reshape(M, P, FREE)
    vg = values.reshape(M, P, GPP)
    tg = target_vals.reshape(M, P, GPP)
    for c in range(M):
        in_maps.append({
            "logits": np.ascontiguousarray(lg[c]),
            "probs": np.ascontiguousarray(pg[c]),
            "vals": np.ascontiguousarray(vg[c]),
            "tvals": np.ascontiguousarray(tg[c]),
        })
    return in_maps


def _finalize(mode, results):
    if mode == "allreduce":
        return np.float32(results[0]["out"][0, 0])
    parts = np.stack([r["out"] for r in results])  # (M, P or 1, 8)
    tot = parts.sum(axis=(0, 1), dtype=np.float64)
    return np.float32((tot[0] + tot[1]) / B)


def kernel(logits, values, target_probs, target_vals, batch_counts):
    from concourse import bass_utils

    logits = np.asarray(logits, dtype=np.float32)
    values = np.asarray(values, dtype=np.float32)
    target_probs = np.asarray(target_probs, dtype=np.float32)
    target_vals = np.asarray(target_vals, dtype=np.float32)
    batch_counts = np.asarray(batch_counts)

    if not (batch_counts.shape == (B,) and np.all(batch_counts == NPG)):
        # Non-uniform segments never occur for this problem's inputs;
        # numpy fallback keeps the contract total.
        return _kernel_numpy(logits, values, target_probs, target_vals,
                             batch_counts)

    nc = _get(MODE)
    in_maps = _make_in_maps(logits, values, target_probs, target_vals)
    last_err = None
    for _ in range(3):
        try:
            res = bass_utils.run_bass_kernel_spmd(
                nc, in_maps, core_ids=list(range(M)))
            return _finalize(MODE, res.results)
        except Exception as e:  # transient runtime/worker hiccup
            last_err = e
    import sys
    print(f"kernel: device run failed ({last_err}); numpy fallback",
          file=sys.stderr)
    return _kernel_numpy(logits, values, target_probs, target_vals,
                         batch_counts)


def _kernel_numpy(logits, values, target_probs, target_vals, batch_counts):
    counts = batch_counts.astype(np.int64)
    b = counts.shape[0]
    idx = np.repeat(np.arange(b), counts)
    loss_val = np.mean((values - target_vals) ** 2, dtype=np.float32)
    probs_sum = target_probs.sum(axis=1)
    lin = (target_probs * logits).sum(axis=1)
    ex = np.exp(logits).sum(axis=1)
    vc = np.zeros(b, np.float32)
    lg = np.zeros(b, np.float32)
    zg = np.zeros(b, np.float32)
    np.add.at(vc, idx, probs_sum)
    np.add.at(lg, idx, lin)
    np.add.at(zg, idx, ex)
    lp = (np.log(zg + EPS) - lg) / (vc + EPS)
    return np.float32(loss_val + lp.mean())


# revision 25
# speedup vs baseline: 1.0373x; 1.0373x over previous
"""AlphaKnotLoss on 8 TRN2 NeuronCores (Bass/Tile, SPMD data-parallel).

Reference computation (B=4096 graphs x 512 nodes x A=10 actions):
  loss_val    = mean((values - target_vals)^2)                  over B
  per graph g: Z[g]   = sum_{n in g, a} exp(logits[n,a])
               Lin[g] = sum_{n in g, a} target_probs[n,a]*logits[n,a]
               V[g]   = sum_{n in g, a} target_probs[n,a]
               lp[g]  = (log(Z[g]+eps) - Lin[g]) / (V[g]+eps)
  out = loss_val + mean(lp)

Sharding: data-parallel over graphs. Each of the 8 cores owns 512 whole
graphs = 262144 nodes. Per-core layout: the (262144, 10) node-major shard
is viewed as (128, 20480) so partition p holds 4 whole graphs
(4 x 512 nodes x 10 actions = 20480 contiguous floats); a graph is a
5120-wide contiguous block of the free axis, split across NT tiles.

Per tile (128 x F): ACT does exp with fused accumulate (Z) and a
copy-accumulate (V); DVE does one fused multiply+reduce via
scalar_tensor_tensor (Lin). The loop is DMA-bound (~56us of HBM traffic
per core); ACT ~37us and DVE ~22us hide underneath it.

Epilogue: per-graph losses on (128, GPP) stats, one PE matmul against a
ones vector for the cross-partition sum, then either a tiny AllReduce
(mode "allreduce": every core returns the final scalar) or per-core
partial sums combined on the host (mode "partials", default: avoids the
collective's mesh-entry barrier, which couples every core to the
slowest/last-started core).
"""

import numpy as np

B = 4096
NPG = 512
N = B * NPG
A = 10
EPS = 1e-9
M = 8  # cores

NC_NODES = N // M          # 262144 nodes per core
NC_GRAPHS = B // M         # 512 graphs per core
P = 128                    # SBUF partitions
FREE = NC_NODES * A // P   # 20480 f32 per partition
GPP = NC_GRAPHS // P       # 4 graphs per partition
GFREE = NPG * A            # 5120 f32 per graph

# Per-graph tile widths (must sum to GFREE). A thin final slice keeps the
# critical path after the last DMA byte short.
GRAPH_TILES = [2560, 2560]
TPG = len(GRAPH_TILES)
NT = GPP * TPG
# When True, the very last graph column uses widths [2560, 1280, 1280] so the
# final DVE/ACT ops after the last DMA byte are half-width.
SPLIT_LAST = True

IO_BUFS = 3
WORK_BUFS = 3

# Engine whose HWDGE queue issues the probs-tile DMAs ("sync" = same queue
# as logits; "tensor" = the otherwise-idle PE sequencer, doubling the
# DMA-issue paths).
PT_ENGINE = "sync"

# False: partials leave via a PE ones-matmul -> PSUM[1,8] -> SBUF -> DRAM.
# True: DMA the per-partition S[128,8] straight out; host reduces.
# A/B (20 reps each): direct-out floor 72.3us vs 68.5us — the 128-row
# 32B-chunk output DMA costs more than the matmul chain. Keep False.
DIRECT_OUT = False

# "lfirst": stream the whole logits shard first (kept resident in SBUF,
# exp/log(Z) finish mid-stream), then stream probs; the post-last-byte
# chain shrinks to one half-width stt + copy + the scalar epilogue.
LFIRST = False


def set_params(graph_tiles=None, io_bufs=None, work_bufs=None, mode=None,
               split_last=None, lfirst=None, direct_out=None,
               pt_engine=None):
    """Tweak build knobs (test harness only); clears the build cache."""
    global GRAPH_TILES, TPG, NT, IO_BUFS, WORK_BUFS, MODE, SPLIT_LAST, LFIRST
    global DIRECT_OUT, PT_ENGINE
    if pt_engine is not None:
        PT_ENGINE = pt_engine
    if split_last is not None:
        SPLIT_LAST = split_last
    if lfirst is not None:
        LFIRST = lfirst
    if direct_out is not None:
        DIRECT_OUT = direct_out
    if graph_tiles is not None:
        assert sum(graph_tiles) == GFREE
        GRAPH_TILES = list(graph_tiles)
        TPG = len(GRAPH_TILES)
        NT = GPP * TPG
    if io_bufs is not None:
        IO_BUFS = io_bufs
    if work_bufs is not None:
        WORK_BUFS = work_bufs
    if mode is not None:
        MODE = mode
    _CACHE.clear()

MODE = "partials"

_CACHE = {}


def _build(mode):
    import concourse.bacc as bacc
    import concourse.mybir as mybir
    import concourse.tile as tile

    f32 = mybir.dt.float32
    Alu = mybir.AluOpType
    Act = mybir.ActivationFunctionType
    AX = mybir.AxisListType.X

    nc = bacc.Bacc("TRN2", target_bir_lowering=False, debug=False,
                   num_devices=M)

    logits = nc.dram_tensor("logits", [P, FREE], f32, kind="ExternalInput")
    probs = nc.dram_tensor("probs", [P, FREE], f32, kind="ExternalInput")
    vals = nc.dram_tensor("vals", [P, GPP], f32, kind="ExternalInput")
    tvals = nc.dram_tensor("tvals", [P, GPP], f32, kind="ExternalInput")
    if mode == "allreduce" or not DIRECT_OUT:
        out = nc.dram_tensor("out", [1, 8], f32, kind="ExternalOutput")
    else:
        # per-partition partials go out directly; host does the final
        # 128x2-per-core reduction (no PE/PSUM in the graph at all)
        out = nc.dram_tensor("out", [P, 8], f32, kind="ExternalOutput")

    import contextlib

    with tile.TileContext(nc) as tc:
        with contextlib.ExitStack() as ctx:
            iop = ctx.enter_context(tc.tile_pool(name="io", bufs=IO_BUFS))
            wp = ctx.enter_context(tc.tile_pool(name="work", bufs=WORK_BUFS))
            sp = ctx.enter_context(tc.tile_pool(name="stats", bufs=1))
            if mode == "allreduce" or not DIRECT_OUT:
                pp = ctx.enter_context(
                    tc.tile_pool(name="psum", bufs=1, space="PSUM"))
            if mode == "allreduce":
                dp = ctx.enter_context(
                    tc.tile_pool(name="dram", bufs=1, space="DRAM"))
            # per-tile widths for each graph column; optionally split the
            # tail of the last graph for a shorter post-DMA chain
            widths = []
            for g in range(GPP):
                if SPLIT_LAST and g == GPP - 1:
                    widths += [GRAPH_TILES[0]]
                    rest = GFREE - GRAPH_TILES[0]
                    widths += [rest // 2, rest - rest // 2]
                else:
                    widths += list(GRAPH_TILES)
            ncols = len(widths)
            Z = sp.tile([P, ncols], f32)
            Lin = sp.tile([P, ncols], f32)
            V = sp.tile([P, ncols], f32)

            fmax = max(GRAPH_TILES)
            off = 0
            for j in range(ncols):
                w = widths[j]
                lt = iop.tile([P, fmax], f32, tag="lt")
                pt = iop.tile([P, fmax], f32, tag="pt")
                et = wp.tile([P, fmax], f32, tag="et")
                cp = wp.tile([P, fmax], f32, tag="cp")
                prod = wp.tile([P, fmax], f32, tag="prod")
                nc.sync.dma_start(lt[:, :w], logits[:, off:off + w])
                pt_eng = {"gpsimd": nc.gpsimd, "scalar": nc.scalar,
                          "sync": nc.sync}[PT_ENGINE]
                pt_eng.dma_start(pt[:, :w], probs[:, off:off + w])
                nc.scalar.activation(et[:, :w], lt[:, :w], Act.Exp,
                                     accum_out=Z[:, j:j + 1])
                nc.scalar.activation(cp[:, :w], pt[:, :w], Act.Copy,
                                     accum_out=V[:, j:j + 1])
                nc.vector.scalar_tensor_tensor(
                    out=prod[:, :w], in0=lt[:, :w], scalar=1.0,
                    in1=pt[:, :w], op0=Alu.mult, op1=Alu.mult,
                    accum_out=Lin[:, j:j + 1])
                off += w

            # reduce tile-partials down to GPP per-graph sums
            nuni = (GPP - 1) * TPG if SPLIT_LAST else GPP * TPG
            Zg = sp.tile([P, GPP], f32)
            Lg = sp.tile([P, GPP], f32)
            Vg = sp.tile([P, GPP], f32)
            for src, dst in ((Z, Zg), (Lin, Lg), (V, Vg)):
                if SPLIT_LAST:
                    nc.vector.reduce_sum(
                        dst[:, 0:GPP - 1],
                        src[:, 0:nuni].rearrange("p (g t) -> p g t", t=TPG),
                        axis=AX)
                    nc.vector.reduce_sum(
                        dst[:, GPP - 1:GPP], src[:, nuni:ncols], axis=AX)
                else:
                    nc.vector.reduce_sum(
                        dst[:, :],
                        src[:, :].rearrange("p (g t) -> p g t", t=TPG),
                        axis=AX)

            # per-graph policy loss: (ln(Z+eps) - Lin) / (V+eps)
            zp = sp.tile([P, GPP], f32)
            nc.vector.tensor_scalar_add(zp[:, :], Zg[:, :], EPS)
            logz = sp.tile([P, GPP], f32)
            nc.scalar.activation(logz[:, :], zp[:, :], Act.Ln)
            num = sp.tile([P, GPP], f32)
            nc.vector.tensor_sub(num[:, :], logz[:, :], Lg[:, :])
            den = sp.tile([P, GPP], f32)
            nc.vector.tensor_scalar_add(den[:, :], Vg[:, :], EPS)
            rec = sp.tile([P, GPP], f32)
            nc.vector.reciprocal(rec[:, :], den[:, :])

            # S[:,0] = per-partition policy sum, S[:,1] = value-sq sum
            S = sp.tile([P, 8], f32)
            nc.gpsimd.memset(S[:, :], 0.0)
            lp = sp.tile([P, GPP], f32)
            nc.vector.scalar_tensor_tensor(
                out=lp[:, :], in0=num[:, :], scalar=1.0, in1=rec[:, :],
                op0=Alu.mult, op1=Alu.mult, accum_out=S[:, 0:1])

            vt = sp.tile([P, GPP], f32)
            tt = sp.tile([P, GPP], f32)
            nc.sync.dma_start(vt[:, :], vals[:, :])
            nc.sync.dma_start(tt[:, :], tvals[:, :])
            d = sp.tile([P, GPP], f32)
            nc.vector.tensor_sub(d[:, :], vt[:, :], tt[:, :])
            d2 = sp.tile([P, GPP], f32)
            nc.vector.scalar_tensor_tensor(
                out=d2[:, :], in0=d[:, :], scalar=1.0, in1=d[:, :],
                op0=Alu.mult, op1=Alu.mult, accum_out=S[:, 1:2])

            if mode == "allreduce" or not DIRECT_OUT:
                # cross-partition sum via matmul with a ones vector
                ones = sp.tile([P, 1], f32)
                nc.gpsimd.memset(ones[:, :], 1.0)
                ps = pp.tile([1, 8], f32)
                nc.tensor.matmul(ps[:, :], ones[:, :], S[:, :],
                                 start=True, stop=True)
                red = sp.tile([1, 8], f32)
                nc.vector.tensor_copy(red[:, :], ps[:, :])
            if mode == "allreduce":
                cin = dp.tile([1, 8], f32)
                cout = dp.tile([1, 8], f32)
                nc.sync.dma_start(cin[:, :], red[:, :])
                nc.gpsimd.collective_compute(
                    "AllReduce", Alu.add,
                    replica_groups=[list(range(M))],
                    ins=[cin[:, :].opt()],
                    outs=[cout[:, :].opt()])
                red2 = sp.tile([1, 8], f32)
                nc.sync.dma_start(red2[:, :], cout[:, :])
                # out = (sum_policy + sum_val) / B
                dummy = sp.tile([1, 2], f32)
                fin = sp.tile([1, 8], f32)
                nc.gpsimd.memset(fin[:, :], 0.0)
                nc.scalar.activation(dummy[:, :], red2[:, 0:2], Act.Copy,
                                     scale=1.0 / B, accum_out=fin[:, 0:1])
                nc.sync.dma_start(out[:, :], fin[:, :])
            elif DIRECT_OUT:
                nc.sync.dma_start(out[:, :], S[:, :])
            else:
                nc.sync.dma_start(out[:, :], red[:, :])

    nc.compile()
    return nc


def _build_lfirst(mode):
    import concourse.bacc as bacc
    import concourse.mybir as mybir
    import concourse.tile as tile

    f32 = mybir.dt.float32
    Alu = mybir.AluOpType
    Act = mybir.ActivationFunctionType
    AX = mybir.AxisListType.X

    nc = bacc.Bacc("TRN2", target_bir_lowering=False, debug=False,
                   num_devices=M)

    logits = nc.dram_tensor("logits", [P, FREE], f32, kind="ExternalInput")
    probs = nc.dram_tensor("probs", [P, FREE], f32, kind="ExternalInput")
    vals = nc.dram_tensor("vals", [P, GPP], f32, kind="ExternalInput")
    tvals = nc.dram_tensor("tvals", [P, GPP], f32, kind="ExternalInput")
    out = nc.dram_tensor("out", [1, 8], f32, kind="ExternalOutput")

    LW = 2560                    # logits tile width
    LNT = FREE // LW             # 8 resident logits tiles
    # probs widths: uniform except the last graph's tail is split in half
    pw = [LW] * (LNT - 1) + [LW // 2, LW // 2]
    PNT = len(pw)                # 9
    LEAD = 3                     # logits tiles ahead of probs in the stream

    with tile.TileContext(nc) as tc:
        with (
            tc.tile_pool(name="lres", bufs=LNT) as lrp,
            tc.tile_pool(name="pio", bufs=IO_BUFS) as pip_,
            tc.tile_pool(name="work", bufs=WORK_BUFS) as wp,
            tc.tile_pool(name="stats", bufs=1) as sp,
            tc.tile_pool(name="psum", bufs=1, space="PSUM") as pp,
        ):
            Z = sp.tile([P, LNT], f32)
            V = sp.tile([P, PNT], f32)
            Lin = sp.tile([P, PNT], f32)

            ltiles = [lrp.tile([P, LW], f32, tag="lt", name=f"lt{j}")
                      for j in range(LNT)]

            def emit_logits(j):
                nc.sync.dma_start(ltiles[j][:, :],
                                  logits[:, j * LW:(j + 1) * LW])
                et = wp.tile([P, LW], f32, tag="et", name=f"et{j}")
                nc.scalar.activation(et[:, :], ltiles[j][:, :], Act.Exp,
                                     accum_out=Z[:, j:j + 1])

            poff = [0]

            def emit_probs(j):
                w = pw[j]
                off = poff[0]
                pt = pip_.tile([P, LW], f32, tag="pt", name=f"pt{j}")
                nc.sync.dma_start(pt[:, :w], probs[:, off:off + w])
                lsrc = ltiles[off // LW][:, off % LW:off % LW + w]
                cp = wp.tile([P, LW], f32, tag="cp", name=f"cp{j}")
                nc.scalar.activation(cp[:, :w], pt[:, :w], Act.Copy,
                                     accum_out=V[:, j:j + 1])
                prod = wp.tile([P, LW], f32, tag="prod", name=f"prod{j}")
                nc.vector.scalar_tensor_tensor(
                    out=prod[:, :w], in0=lsrc, scalar=1.0, in1=pt[:, :w],
                    op0=Alu.mult, op1=Alu.mult,
                    accum_out=Lin[:, j:j + 1])
                poff[0] += w

            # interleaved stream: logits LEAD tiles ahead so exp/log(Z)
            # finish before the probs stream ends
            li = pi = 0
            for j in range(LEAD):
                emit_logits(li)
                li += 1
            while li < LNT:
                emit_probs(pi)
                pi += 1
                emit_logits(li)
                li += 1
            # log(Z+eps) per graph — scheduled right after the last exp,
            # well before the stream ends
            Zg = sp.tile([P, GPP], f32)
            nc.vector.reduce_sum(
                Zg[:, :], Z[:, :].rearrange("p (g t) -> p g t", t=2),
                axis=AX)
            zp = sp.tile([P, GPP], f32)
            nc.vector.tensor_scalar_add(zp[:, :], Zg[:, :], EPS)
            logz = sp.tile([P, GPP], f32)
            nc.scalar.activation(logz[:, :], zp[:, :], Act.Ln)
            while pi < PNT:
                emit_probs(pi)
                pi += 1

            # per-graph sums: graphs 0..2 from column pairs, graph 3 from
            # the last three columns
            Vg = sp.tile([P, GPP], f32)
            Lg = sp.tile([P, GPP], f32)
            for src, dst in ((V, Vg), (Lin, Lg)):
                nc.vector.reduce_sum(
                    dst[:, 0:GPP - 1],
                    src[:, 0:2 * (GPP - 1)].rearrange(
                        "p (g t) -> p g t", t=2),
                    axis=AX)
                nc.vector.reduce_sum(dst[:, GPP - 1:GPP],
                                     src[:, 2 * (GPP - 1):PNT], axis=AX)

            den = sp.tile([P, GPP], f32)
            nc.vector.tensor_scalar_add(den[:, :], Vg[:, :], EPS)
            rec = sp.tile([P, GPP], f32)
            nc.vector.reciprocal(rec[:, :], den[:, :])
            num = sp.tile([P, GPP], f32)
            nc.vector.tensor_sub(num[:, :], logz[:, :], Lg[:, :])

            S = sp.tile([P, 8], f32)
            nc.gpsimd.memset(S[:, :], 0.0)
            lp = sp.tile([P, GPP], f32)
            nc.vector.scalar_tensor_tensor(
                out=lp[:, :], in0=num[:, :], scalar=1.0, in1=rec[:, :],
                op0=Alu.mult, op1=Alu.mult, accum_out=S[:, 0:1])

            vt = sp.tile([P, GPP], f32)
            tt = sp.tile([P, GPP], f32)
            nc.sync.dma_start(vt[:, :], vals[:, :])
            nc.sync.dma_start(tt[:, :], tvals[:, :])
            d = sp.tile([P, GPP], f32)
            nc.vector.tensor_sub(d[:, :], vt[:, :], tt[:, :])
            d2 = sp.tile([P, GPP], f32)
            nc.vector.scalar_tensor_tensor(
                out=d2[:, :], in0=d[:, :], scalar=1.0, in1=d[:, :],
                op0=Alu.mult, op1=Alu.mult, accum_out=S[:, 1:2])

            ones = sp.tile([P, 1], f32)
            nc.gpsimd.memset(ones[:, :], 1.0)
            ps = pp.tile([1, 8], f32)
            nc.tensor.matmul(ps[:, :], ones[:, :], S[:, :],
                             start=True, stop=True)
            red = sp.tile([1, 8], f32)
            nc.vector.tensor_copy(red[:, :], ps[:, :])
            nc.sync.dma_start(out[:, :], red[:, :])

    nc.compile()
    return nc


def _get(mode):
    if mode not in _CACHE:
        _CACHE[mode] = (_build_lfirst(mode) if LFIRST and mode == "partials"
                        else _build(mode))
    return _CACHE[mode]


def _make_in_maps(logits, values, target_probs, target_vals):
    in_maps = []
    lg = logits.reshape(M, P, FREE)
    pg = target_probs.reshape(M, P, FREE)
    vg = values.reshape(M, P, GPP)
    tg = target_vals.reshape(M, P, GPP)
    for c in range(M):
        in_maps.append({
            "logits": np.ascontiguousarray(lg[c]),
            "probs": np.ascontiguousarray(pg[c]),
            "vals": np.ascontiguousarray(vg[c]),
            "tvals": np.ascontiguousarray(tg[c]),
        })
    return in_maps


def _finalize(mode, results):
    if mode == "allreduce":
        return np.float32(results[0]["out"][0, 0])
    parts = np.stack([r["out"] for r in results])  # (M, P or 1, 8)
    tot = parts.sum(axis=(0, 1), dtype=np.float64)
    return np.float32((tot[0] + tot[1]) / B)


def kernel(logits, values, target_probs, target_vals, batch_counts):
    from concourse import bass_utils

    logits = np.asarray(logits, dtype=np.float32)
    values = np.asarray(values, dtype=np.float32)
    target_probs = np.asarray(target_probs, dtype=np.float32)
    target_vals = np.asarray(target_vals, dtype=np.float32)
    batch_counts = np.asarray(batch_counts)

    if not (batch_counts.shape == (B,) and np.all(batch_counts == NPG)):
        # Non-uniform segments never occur for this problem's inputs;
        # numpy fallback keeps the contract total.
        return _kernel_numpy(logits, values, target_probs, target_vals,
                             batch_counts)

    nc = _get(MODE)
    in_maps = _make_in_maps(logits, values, target_probs, target_vals)
    last_err = None
    for _ in range(3):
        try:
            res = bass_utils.run_bass_kernel_spmd(
                nc, in_maps, core_ids=list(range(M)))
            return _finalize(MODE, res.results)
        except Exception as e:  # transient runtime/worker hiccup
            last_err = e
    import sys
    print(f"kernel: device run failed ({last_err}); numpy fallback",
          file=sys.stderr)
    return _kernel_numpy(logits, values, target_probs, target_vals,
                         batch_counts)


def _kernel_numpy(logits, values, target_probs, target_vals, batch_counts):
    counts = batch_counts.astype(np.int64)
    b = counts.shape[0]
    idx = np.repeat(np.arange(b), counts)
    loss_val = np.mean((values - target_vals) ** 2, dtype=np.float32)
    probs_sum = target_probs.sum(axis=1)
    lin = (target_probs * logits).sum(axis=1)
    ex = np.exp(logits).sum(axis=1)
    vc = np.zeros(b, np.float32)
    lg = np.zeros(b, np.float32)
    zg = np.zeros(b, np.float32)
    np.add.at(vc, idx, probs_sum)
    np.add.at(lg, idx, lin)
    np.add.at(zg, idx, ex)
    lp = (np.log(zg + EPS) - lg) / (vc + EPS)
    return np.float32(loss_val + lp.mean())
